# revision 4
# baseline (speedup 1.0000x reference)
"""Trainium2 Bass kernel for the 3-layer GAT model (nn_GATModel_71777493450787).

Strategy (8 NeuronCores, SPMD single program):
  - Nodes relabeled by in-degree round-robin over 392 blocks (balances edges
    per 128-node dst block), padded to NPAD = 8*49*128, range-partitioned by
    destination block across cores.
  - Per layer each core computes a 256-byte packed row per owned node:
    [hp fp8(128B) | a_s bf16(8B) | fp8 residual of hp[0:120] (120B)], one
    matmul per block (att vectors folded into weight columns), AllGathered
    into a shared [NPAD, 256B] table.
  - Edges grouped per dst block, sorted by src, split into lo/hi halves
    (src < 32768 vs >=) so indices fit int16; per-edge rows fetched with ONE
    dma_gather per 4-block group per half (~6.8 ns/edge, the SWDGE wall).
    hp is reconstructed as fp8 + residual (bf16-equivalent precision).
  - a_d[dst] per edge via per-tile one-hot transposes + tiny matmuls on the
    otherwise idle PE/ACT engines; self-loop injected with an identity
    matmul from local data (PyG fill_value='mean' handled by a host-computed
    a_e_loop table).
  - Scatter-add via one-hot matmuls accumulating [agg | denom] in PSUM;
    softmax without max-subtraction (|alpha| < ~6 for this model).
  - Final mean + 2-layer MLP on-device; result of core 0 returned.
"""
import numpy as np
import ml_dtypes

import concourse.bass as bass
import concourse.bacc as bacc
import concourse.mybir as mybir
import concourse.tile as tile
import concourse.ap_utils as ap_utils
from concourse.bass_utils import run_bass_kernel_spmd

BF16 = np.dtype(ml_dtypes.bfloat16)
FP8 = np.dtype(ml_dtypes.float8_e4m3)
FP32 = mybir.dt.float32
BF = mybir.dt.bfloat16
F8 = mybir.dt.float8e4
I16 = mybir.dt.int16

P = 128
H = 4
C = 32
F = 128           # H*C
ED = 16
L = 3
NEG = 0.2
NCORES = 8
RB = 256          # table row bytes (fp8 elems)
NODE_COLS = F + 8   # node-phase matmul out: hp(128)+a_s(4)+a_d(4)
HALF = 32768
GB = 4            # dst blocks per dma_gather
RESD = 120        # residual-covered features


def _wrap16(flat):
    """flat int array -> [128, ceil(n/16)] int16: index j at (j%16, j//16),
    replicated to all 128 partitions."""
    n = len(flat)
    cols = (n + 15) // 16
    a = np.zeros((16, cols), np.int16)
    a[np.arange(n) % 16, np.arange(n) // 16] = flat.astype(np.int16)
    return np.tile(a, (8, 1))


def _dma_gather_thin(gp, out_ap, in_ap, idxs_ap, num_idxs, elem_size,
                     elem_step):
    """dma_gather without the elem%256 restriction (non-transpose path only
    needs the row stride to be a 256B multiple)."""
    from concourse.bass import exact_div
    assert idxs_ap.dtype == mybir.dt.int16
    assert in_ap.dtype == out_ap.dtype
    assert ap_utils.ap_is_contiguous(out_ap.ap[1:])
    assert ap_utils.ap_is_contiguous(idxs_ap.ap[1:])
    assert in_ap.ap[-1][1] == out_ap.ap[-1][1] == elem_size
    assert out_ap.ap[0][1] * out_ap.ap[1][1] == ((num_idxs + 127) // 128) * 128
    assert in_ap.ap[0][0] == elem_step
    stride_bytes = elem_step * mybir.dt.size(in_ap.dtype)
    stride_bytes_256 = exact_div(stride_bytes, 256)
    _in_ap = gp.lower_ap_dma(in_ap, for_custom_bir_dma=True)
    _idxs_ap = gp.lower_ap(idxs_ap)
    _out_ap = gp.lower_ap(out_ap)
    return gp.add_instruction(
        mybir.InstDMAGatherAnt(
            name=gp.bass.get_next_instruction_name(),
            ins=[*_in_ap, _idxs_ap, gp.lower_val_access(gp.to_reg(num_idxs))],
            outs=[_out_ap],
            transpose=False, num_idxs=num_idxs, elem_size=elem_size,
            stride_bytes_256=stride_bytes_256, gen_mode=0,
            single_packet=False, queue_num=0, sbuf_tokens_per_rank=0,
            sbuf_free_dim_per_rank=0, sbuf_free_dim_pad_per_rank=0,
            sbuf_byte_offset=0,
        ))


# ---------------------------------------------------------------- host prep
def host_prep(inputs, ncores=NCORES):
    src0 = np.asarray(inputs["edge_index"])[0].astype(np.int64)
    dst0 = np.asarray(inputs["edge_index"])[1].astype(np.int64)
    ea = np.asarray(inputs["edge_attr"]).astype(np.float32)
    x0 = np.asarray(inputs["x"]).astype(np.float32)
    n_nodes, n_edges = x0.shape[0], src0.shape[0]

    nb = int(np.ceil(n_nodes / (ncores * P)))
    npad = ncores * nb * P
    nblk = npad // P

    # degree-balanced relabeling: rank r (in-degree desc) -> block r%nblk
    indeg = np.bincount(dst0, minlength=n_nodes)
    rank = np.argsort(-indeg, kind="stable")       # rank -> old id
    new_of_rank = (np.arange(n_nodes) % nblk) * P + np.arange(n_nodes) // nblk
    old2new = np.empty(n_nodes, np.int64)
    old2new[rank] = new_of_rank
    src = old2new[src0]
    dst = old2new[dst0]
    xpad = np.zeros((npad, F), np.float32)
    xpad[old2new] = x0
    mask = np.zeros(npad, np.float32)
    mask[old2new] = 1.0

    # static edge attention term a_e[e, l*4+h]
    W_edge = np.asarray(inputs["W_edge"], np.float32).reshape(L, ED, H, C)
    att_edge = np.asarray(inputs["att_edge"], np.float32)
    u_e = np.einsum("ldhc,lhc->ldh", W_edge, att_edge)
    a_e = np.einsum("ed,ldh->elh", ea, u_e).reshape(n_edges, L * H)

    deg = np.bincount(dst, minlength=npad).astype(np.float32)
    a_e_loop = np.zeros((npad, L * H), np.float32)
    np.add.at(a_e_loop, dst, a_e)
    a_e_loop /= np.maximum(deg, 1.0)[:, None]

    # per-block lo/hi slot layout, sorted by src
    blk = dst // P
    half = (src >= HALF).astype(np.int64)
    order = np.lexsort((src, half, blk))
    src_s, dst_s, ae_s, half_s = src[order], dst[order], a_e[order], half[order]
    blk_s = blk[order]
    cnt_lo = np.bincount(blk_s[half_s == 0], minlength=nblk)
    cnt_hi = np.bincount(blk_s[half_s == 1], minlength=nblk)
    s_lo = int(np.ceil(cnt_lo.max() / P))
    s_hi = int(np.ceil(cnt_hi.max() / P))
    stot = s_lo + s_hi

    idx_lo = np.zeros((nblk, s_lo * P), np.int64)
    idx_hi = np.zeros((nblk, s_hi * P), np.int64)
    dl_a = np.full((nblk, stot * P), 200.0, np.float32)  # lo tiles then hi
    ae_a = np.zeros((nblk, stot * P, L * H), np.float32)
    starts = np.zeros(nblk + 1, np.int64)
    np.cumsum(cnt_lo + cnt_hi, out=starts[1:])
    for b in range(nblk):
        s = starts[b]
        nlo, nhi = cnt_lo[b], cnt_hi[b]
        idx_lo[b, :nlo] = src_s[s:s + nlo]
        dl_a[b, :nlo] = (dst_s[s:s + nlo] - b * P).astype(np.float32)
        ae_a[b, :nlo] = ae_s[s:s + nlo]
        o = s_lo * P
        idx_hi[b, :nhi] = src_s[s + nlo:s + nlo + nhi] - HALF
        dl_a[b, o:o + nhi] = (dst_s[s + nlo:s + nlo + nhi] - b * P)
        ae_a[b, o:o + nhi] = ae_s[s + nlo:s + nlo + nhi]

    def dev_slot(a, s):   # [nb, s*128(,d)] -> [128, nb*s(,d)]
        d = a.shape[2:] if a.ndim == 3 else ()
        a = a.reshape(nb, s, P, *d)
        a = np.moveaxis(a, 2, 0)
        return np.ascontiguousarray(a.reshape(P, nb * s, *d))

    # node-phase weights [W | u_s | u_d] per layer
    W_src = np.asarray(inputs["W_src"], np.float32)
    att_src = np.asarray(inputs["att_src"], np.float32)
    att_dst = np.asarray(inputs["att_dst"], np.float32)
    u_s = np.einsum("lkhc,lhc->lkh", W_src.reshape(L, F, H, C), att_src)
    u_d = np.einsum("lkhc,lhc->lkh", W_src.reshape(L, F, H, C), att_dst)
    W3ext = np.concatenate(
        [np.concatenate([W_src[l], u_s[l], u_d[l]], axis=1)
         for l in range(L)], axis=1)               # [128, L*136]

    bias = np.asarray(inputs["bias"], np.float32)
    bias_rep = np.tile(bias.reshape(1, L * F), (P, 1))
    W1 = np.asarray(inputs["W1"], np.float32)
    b1 = np.asarray(inputs["b1"], np.float32).reshape(1, 2 * F)
    W2 = np.asarray(inputs["W2"], np.float32)
    W2ab = np.concatenate([W2[:F], W2[F:]], axis=1)
    b2 = np.asarray(inputs["b2"], np.float32).reshape(1, 2)

    cores = []
    for c in range(ncores):
        bs = slice(c * nb, (c + 1) * nb)
        gbsl = slice(c * nb * P, (c + 1) * nb * P)
        # gather index streams: flat order (block, tile, partition), %16-safe
        il = idx_lo[bs].reshape(-1)
        ih = idx_hi[bs].reshape(-1)
        dlc = dl_a[bs]                                        # [nb, stot*128]
        ohT = (dlc[None, :, :] == np.arange(P, dtype=np.float32)[
            :, None, None]).astype(FP8)                       # [128d, nb, stot*128]
        cores.append({
            "idx16_lo": _wrap16(il),                              # [128, nb*s_lo*8]
            "idx16_hi": _wrap16(ih),
            "ohT_all": np.ascontiguousarray(ohT.reshape(P, -1)), # [128, nb*stot*128]
            "dstloc": dev_slot(dl_a[bs], stot).astype(BF16),      # [128, nb*stot]
            "a_e_all": dev_slot(ae_a[bs], stot).reshape(P, -1).astype(BF16),
            "a_e_loop": np.ascontiguousarray(
                a_e_loop[gbsl].reshape(nb, P, L * H).transpose(1, 0, 2)
                .reshape(P, nb * L * H)).astype(BF16),
            "x_all": np.ascontiguousarray(
                xpad[gbsl].reshape(nb, P, F).transpose(1, 0, 2)
                .reshape(P, nb * F)).astype(BF16),
            "mask": np.ascontiguousarray(
                mask[gbsl].reshape(nb, P).T).astype(BF16),
            "W3ext": W3ext.astype(BF16),
            "bias_rep": bias_rep.astype(np.float32),
            "W1": W1.astype(BF16), "b1": b1, "W2ab": W2ab.astype(BF16),
            "b2": b2,
        })
    return dict(cores=cores, nb=nb, ktiles=s_lo * 1000 + s_hi, npad=npad,
                n_nodes=n_nodes)


def make_in_maps(prep, ncores=NCORES):
    keys = ["idx16_lo", "idx16_hi", "ohT_all", "dstloc", "a_e_all", "a_e_loop", "x_all",
            "mask", "W3ext", "bias_rep", "W1", "b1", "W2ab", "b2"]
    return [{k: prep["cores"][c][k] for k in keys} for c in range(ncores)]


# ------------------------------------------------------------ program build
def build_program(nb, ktiles, npad, n_nodes, ncores=NCORES, reps=1,
                  no_collective=False, variant="full"):
    from concourse.masks import make_identity
    S_LO, S_HI = ktiles // 1000, ktiles % 1000
    ST = S_LO + S_HI
    NB = nb
    ngrp = (NB + GB - 1) // GB
    nc = bacc.Bacc("TRN2", target_bir_lowering=False, num_devices=ncores)

    ti = {}
    def ext(name, shape, dtype):
        ti[name] = nc.dram_tensor(name, shape, dtype, kind="ExternalInput")
    ext("idx16_lo", [P, NB * S_LO * 8], I16)
    ext("idx16_hi", [P, NB * S_HI * 8], I16)
    ext("ohT_all", [P, NB * ST * P], F8)
    ext("dstloc", [P, NB * ST], BF)
    ext("a_e_all", [P, NB * ST * L * H], BF)
    ext("a_e_loop", [P, NB * L * H], BF)
    ext("x_all", [P, NB * F], BF)
    ext("mask", [P, NB], BF)
    ext("W3ext", [F, L * NODE_COLS], BF)
    ext("bias_rep", [P, L * F], FP32)
    ext("W1", [F, 2 * F], BF)
    ext("b1", [1, 2 * F], FP32)
    ext("W2ab", [F, 4], BF)
    ext("b2", [1, 2], FP32)

    y = nc.dram_tensor("y", [1, 2], FP32, kind="ExternalOutput")
    slice_dram = nc.dram_tensor("slice_dram", [NB * P, RB], F8)
    table = nc.dram_tensor("table", [npad, RB], F8, addr_space="Shared")
    g_in = nc.dram_tensor("g_in", [1, F], FP32)
    g_out = nc.dram_tensor("g_out", [1, F], FP32, addr_space="Shared")
    groups = [list(range(ncores))]

    from contextlib import ExitStack
    with tile.TileContext(nc) as tc, ExitStack() as ctx:
        cpool = ctx.enter_context(tc.tile_pool(name="const", bufs=1))
        bpool = ctx.enter_context(tc.tile_pool(name="bundle", bufs=1))
        glop = ctx.enter_context(tc.tile_pool(name="glo", bufs=2))
        ghip = ctx.enter_context(tc.tile_pool(name="ghi", bufs=2))
        rpool = ctx.enter_context(tc.tile_pool(name="recon", bufs=2))
        mpool = ctx.enter_context(tc.tile_pool(name="msg", bufs=2))
        opool = ctx.enter_context(tc.tile_pool(name="onehot", bufs=2))
        tpool = ctx.enter_context(tc.tile_pool(name="ohT", bufs=2))
        apool = ctx.enter_context(tc.tile_pool(name="alpha", bufs=2))
        spool = ctx.enter_context(tc.tile_pool(name="small", bufs=4))
        npool = ctx.enter_context(tc.tile_pool(name="node", bufs=3))
        pag = ctx.enter_context(tc.tile_pool(name="pag", bufs=2, space="PSUM"))
        ptr = ctx.enter_context(tc.tile_pool(name="ptr", bufs=1, space="PSUM"))
        padg = ctx.enter_context(tc.tile_pool(name="padg", bufs=2, space="PSUM"))
        pnode = ctx.enter_context(tc.tile_pool(name="pnode", bufs=2, space="PSUM"))
        pg = ctx.enter_context(tc.tile_pool(name="pg", bufs=1, space="PSUM"))

        def load(name, shape, dtype):
            t = cpool.tile(shape, dtype, tag=name)
            nc.sync.dma_start(t[:], ti[name][:])
            return t
        idx16_lo = load("idx16_lo", [P, NB * S_LO * 8], I16)
        idx16_hi = load("idx16_hi", [P, NB * S_HI * 8], I16)
        dstloc = load("dstloc", [P, NB * ST], BF)
        a_e_all = load("a_e_all", [P, NB * ST, L * H], BF)
        a_e_loop = load("a_e_loop", [P, NB, L * H], BF)
        x_all = load("x_all", [P, NB, F], BF)
        maskt = load("mask", [P, NB], BF)
        W3ext = load("W3ext", [F, L * NODE_COLS], BF)
        bias_rep = load("bias_rep", [P, L * F], FP32)
        W1 = load("W1", [F, 2 * F], BF)
        b1 = load("b1", [1, 2 * F], FP32)
        W2ab = load("W2ab", [F, 4], BF)
        b2 = load("b2", [1, 2], FP32)

        ident = cpool.tile([P, P], BF)
        make_identity(nc, ident[:])
        iota32 = cpool.tile([P, P], mybir.dt.int32)
        nc.gpsimd.iota(iota32[:], pattern=[[1, P]], base=0,
                       channel_multiplier=0)
        iota_bf = cpool.tile([P, P], BF)
        nc.vector.tensor_copy(iota_bf[:], iota32[:])

        bundle8 = bpool.tile([P, NB, RB], F8)
        a_d_all = bpool.tile([P, NB, 4], BF)
        a_d_hi = bpool.tile([P, NB, 4], F8)
        a_d_lo = bpool.tile([P, NB, 4], F8)

        # ---- node phase: src_ap [128n, 128f] bf16 -> layer-l packed row
        def node_phase(src_ap, l, b):
            srcT_ps = ptr.tile([P, P], BF, tag="tr")
            nc.tensor.transpose(out=srcT_ps[:], in_=src_ap, identity=ident[:])
            srcT = npool.tile([P, P], BF, tag="srcT")
            nc.any.tensor_copy(srcT[:], srcT_ps[:])
            nb_ps = pnode.tile([P, NODE_COLS], FP32, tag="np")
            nc.tensor.matmul(nb_ps[:], lhsT=srcT[:],
                             rhs=W3ext[:, l * NODE_COLS:(l + 1) * NODE_COLS],
                             start=True, stop=True)
            nc.any.tensor_copy(bundle8[:, b, 0:F], nb_ps[:, 0:F])
            nc.vector.tensor_tensor(
                out=bundle8[:, b, F + 8:F + 8 + RESD],
                in0=nb_ps[:, 0:RESD], in1=bundle8[:, b, 0:RESD],
                op=mybir.AluOpType.subtract)
            nc.any.tensor_copy(bundle8[:, b, F:F + 8].bitcast(BF),
                               nb_ps[:, F:F + 4])
            nc.any.tensor_copy(a_d_all[:, b, :], nb_ps[:, F + 4:F + 8])
            nc.any.tensor_copy(a_d_hi[:, b, :], nb_ps[:, F + 4:F + 8])
            nc.vector.tensor_tensor(
                out=a_d_lo[:, b, :], in0=nb_ps[:, F + 4:F + 8],
                in1=a_d_hi[:, b, :], op=mybir.AluOpType.subtract)
            nc.sync.dma_start(slice_dram[b * P:(b + 1) * P, :],
                              bundle8[:, b, :])

        def reconstruct(gsl, s_n, tag):
            """gathered [P, s_n, RB] fp8 -> bf16 hp [P, s_n, F]."""
            r = rpool.tile([P, s_n, F], BF, tag=tag)
            nc.vector.tensor_tensor(
                out=r[:, :, 0:RESD], in0=gsl[:, :, 0:RESD],
                in1=gsl[:, :, F + 8:F + 8 + RESD], op=mybir.AluOpType.add)
            nc.any.tensor_copy(r[:, :, RESD:F], gsl[:, :, RESD:F])
            return r

        # ---- edge phase for (block b, layer l)
        def edge_phase(b, l, g_lo, g_hi, ohTg, goff):
            gl = g_lo[:, goff * S_LO:(goff + 1) * S_LO, :]
            gh = g_hi[:, goff * S_HI:(goff + 1) * S_HI, :]
            rlo = reconstruct(gl, S_LO, "rlo")
            rhi = reconstruct(gh, S_HI, "rhi")
            oh = opool.tile([P, ST, P], BF, tag="oh")
            nc.vector.tensor_tensor(
                out=oh[:],
                in0=dstloc[:, b * ST:(b + 1) * ST][:, :, None].to_broadcast(
                    [P, ST, P]),
                in1=iota_bf[:, None, :].to_broadcast([P, ST, P]),
                op=mybir.AluOpType.is_equal)
            adg_ps = padg.tile([P, ST, 4], FP32)
            if variant != "noadg":
                for t in range(ST):
                    sl = ohTg[:, (goff * ST + t) * P:(goff * ST + t + 1) * P]
                    nc.tensor.matmul(adg_ps[:, t, :], lhsT=sl,
                                     rhs=a_d_hi[:, b, :],
                                     start=True, stop=False)
                    nc.tensor.matmul(adg_ps[:, t, :], lhsT=sl,
                                     rhs=a_d_lo[:, b, :],
                                     start=False, stop=True)
            else:
                nc.vector.memset(adg_ps[:], 0.0)
            alpha = apool.tile([P, ST, 4], FP32, tag="al")
            nc.vector.tensor_tensor(
                out=alpha[:, 0:S_LO, :], in0=gl[:, :, F:F + 8].bitcast(BF),
                in1=a_e_all[:, b * ST:b * ST + S_LO, l * H:(l + 1) * H],
                op=mybir.AluOpType.add)
            nc.vector.tensor_tensor(
                out=alpha[:, S_LO:, :], in0=gh[:, :, F:F + 8].bitcast(BF),
                in1=a_e_all[:, b * ST + S_LO:(b + 1) * ST, l * H:(l + 1) * H],
                op=mybir.AluOpType.add)
            alpha2 = apool.tile([P, ST, 4], FP32, tag="al2")
            nc.vector.tensor_tensor(out=alpha2[:], in0=alpha[:],
                                    in1=adg_ps[:], op=mybir.AluOpType.add)
            lrt = apool.tile([P, ST, 4], FP32, tag="lrt")
            nc.vector.tensor_scalar(out=lrt[:], in0=alpha2[:], scalar1=NEG,
                                    scalar2=None, op0=mybir.AluOpType.mult)
            lr = apool.tile([P, ST, 4], FP32, tag="lr")
            nc.vector.tensor_tensor(out=lr[:], in0=alpha2[:], in1=lrt[:],
                                    op=mybir.AluOpType.max)
            msg = mpool.tile([P, ST, F + 4], BF, tag="msg")
            nc.scalar.activation(msg[:, :, F:F + 4], lr[:],
                                 mybir.ActivationFunctionType.Exp)
            nc.vector.tensor_tensor(
                out=msg[:, 0:S_LO, 0:F].rearrange("p s (h c) -> p s h c", h=H),
                in0=rlo[:].rearrange("p s (h c) -> p s h c", h=H),
                in1=msg[:, 0:S_LO, F:F + 4][:, :, :, None].to_broadcast(
                    [P, S_LO, H, C]),
                op=mybir.AluOpType.mult)
            nc.vector.tensor_tensor(
                out=msg[:, S_LO:, 0:F].rearrange("p s (h c) -> p s h c", h=H),
                in0=rhi[:].rearrange("p s (h c) -> p s h c", h=H),
                in1=msg[:, S_LO:, F:F + 4][:, :, :, None].to_broadcast(
                    [P, S_HI, H, C]),
                op=mybir.AluOpType.mult)
            # self-loop message from local bundle
            rown = npool.tile([P, F], BF, tag="rown")
            nc.vector.tensor_tensor(
                out=rown[:, 0:RESD], in0=bundle8[:, b, 0:RESD],
                in1=bundle8[:, b, F + 8:F + 8 + RESD], op=mybir.AluOpType.add)
            nc.any.tensor_copy(rown[:, RESD:F], bundle8[:, b, RESD:F])
            t1 = spool.tile([P, 4], FP32, tag="t1")
            nc.vector.tensor_tensor(out=t1[:],
                                    in0=bundle8[:, b, F:F + 8].bitcast(BF),
                                    in1=a_d_all[:, b, :],
                                    op=mybir.AluOpType.add)
            t2 = spool.tile([P, 4], FP32, tag="t2")
            nc.vector.tensor_tensor(
                out=t2[:], in0=t1[:],
                in1=a_e_loop[:, b, l * H:(l + 1) * H], op=mybir.AluOpType.add)
            lrlt = spool.tile([P, 4], FP32, tag="lrlt")
            nc.vector.tensor_scalar(out=lrlt[:], in0=t2[:], scalar1=NEG,
                                    scalar2=None, op0=mybir.AluOpType.mult)
            lrl = spool.tile([P, 4], FP32, tag="lrl")
            nc.vector.tensor_tensor(out=lrl[:], in0=t2[:], in1=lrlt[:],
                                    op=mybir.AluOpType.max)
            msl = mpool.tile([P, F + 4], BF, tag="msl")
            nc.scalar.activation(msl[:, F:F + 4], lrl[:],
                                 mybir.ActivationFunctionType.Exp)
            nc.vector.tensor_tensor(
                out=msl[:, 0:F].rearrange("p (h c) -> p h c", h=H),
                in0=rown[:].rearrange("p (h c) -> p h c", h=H),
                in1=msl[:, F:F + 4][:, :, None].to_broadcast([P, H, C]),
                op=mybir.AluOpType.mult)
            # aggregate
            agg = pag.tile([P, F + 4], FP32)
            nc.tensor.matmul(agg[:], lhsT=ident[:], rhs=msl[:],
                             start=True, stop=False)
            for t in range(ST):
                nc.tensor.matmul(agg[:], lhsT=oh[:, t, :], rhs=msg[:, t, :],
                                 start=False, stop=(t == ST - 1))
            den = spool.tile([P, 4], FP32, tag="den")
            nc.vector.tensor_scalar(out=den[:], in0=agg[:, F:F + 4],
                                    scalar1=1e-30, scalar2=None,
                                    op0=mybir.AluOpType.max)
            rec = spool.tile([P, 4], FP32, tag="rec")
            nc.vector.reciprocal(rec[:], den[:])
            hval = npool.tile([P, F], FP32, tag="hval")
            nc.vector.tensor_tensor(
                out=hval[:].rearrange("p (h c) -> p h c", h=H),
                in0=agg[:, 0:F].rearrange("p (h c) -> p h c", h=H),
                in1=rec[:][:, :, None].to_broadcast([P, H, C]),
                op=mybir.AluOpType.mult)
            hb = npool.tile([P, F], FP32, tag="hb")
            nc.vector.tensor_tensor(out=hb[:], in0=hval[:],
                                    in1=bias_rep[:, l * F:(l + 1) * F],
                                    op=mybir.AluOpType.add)
            h_new = npool.tile([P, F], BF, tag="h_new")
            nc.vector.tensor_scalar(out=h_new[:], in0=hb[:], scalar1=0.0,
                                    scalar2=None, op0=mybir.AluOpType.max)
            return h_new

        def allgather():
            if no_collective:
                for c in range(ncores):
                    nc.sync.dma_start(
                        table[c * NB * P:(c + 1) * NB * P, :], slice_dram[:])
                return
            nc.gpsimd.collective_compute(
                "AllGather", mybir.AluOpType.bypass, replica_groups=groups,
                ins=[slice_dram[:]], outs=[table[:]])

        # ---- main flow
        rep_cm = tc.For_i(0, reps, 1) if reps > 1 else None
        if rep_cm is not None:
            rep_cm.__enter__()
        for b in range(NB):
            node_phase(x_all[:, b, :], 0, b)
        allgather()
        g_acc = cpool.tile([1, F], FP32)
        nc.vector.memset(g_acc[:], 0.0)
        for l in range(L):
            for grp in range(ngrp):
                b0 = grp * GB
                gbn = min(GB, NB - b0)
                g_lo = glop.tile([P, GB * S_LO, RB], F8, tag="g_lo")
                g_hi = ghip.tile([P, GB * S_HI, RB], F8, tag="g_hi")
                if variant != "nogather":
                    _dma_gather_thin(
                        nc.gpsimd, g_lo[:, :gbn * S_LO, :], table[0:HALF, :],
                        idx16_lo[:, b0 * S_LO * 8:(b0 + gbn) * S_LO * 8],
                        gbn * S_LO * P, RB, RB)
                    _dma_gather_thin(
                        nc.gpsimd, g_hi[:, :gbn * S_HI, :], table[HALF:, :],
                        idx16_hi[:, b0 * S_HI * 8:(b0 + gbn) * S_HI * 8],
                        gbn * S_HI * P, RB, RB)
                else:
                    nc.vector.memset(g_lo[:, 0, 0:4], 0.0)
                    nc.vector.memset(g_hi[:, 0, 0:4], 0.0)
                ohTg = tpool.tile([P, GB * ST * P], F8, tag="ohTg")
                nc.sync.dma_start(
                    ohTg[:, :gbn * ST * P],
                    ti["ohT_all"][:, b0 * ST * P:(b0 + gbn) * ST * P])
                if variant == "gatheronly":
                    sink = spool.tile([P, 4], FP32, tag="sink")
                    nc.vector.tensor_tensor(
                        out=sink[:], in0=g_lo[:, 0, 0:4],
                        in1=g_hi[:, 0, 0:4], op=mybir.AluOpType.add)
                    continue
                for goff in range(gbn):
                    b = b0 + goff
                    h_new = edge_phase(b, l, g_lo, g_hi, ohTg, goff)
                    if l < L - 1:
                        node_phase(h_new[:], l + 1, b)
                    else:
                        gblk = pg.tile([1, F], FP32, tag="gblk")
                        nc.tensor.matmul(gblk[:], lhsT=maskt[:, b:b + 1],
                                         rhs=h_new[:], start=True, stop=True)
                        nc.vector.tensor_tensor(out=g_acc[:], in0=g_acc[:],
                                                in1=gblk[:],
                                                op=mybir.AluOpType.add)
            if variant == "gatheronly" and l < L - 1:
                for b in range(NB):
                    node_phase(x_all[:, b, :], l + 1, b)
            if l < L - 1:
                allgather()

        # ---- mean + MLP (redundant on every core)
        g_sb = spool.tile([1, F], FP32, tag="g_sb")
        nc.vector.tensor_scalar(out=g_sb[:], in0=g_acc[:],
                                scalar1=1.0 / n_nodes, scalar2=None,
                                op0=mybir.AluOpType.mult)
        nc.sync.dma_start(g_in[:], g_sb[:])
        if no_collective:
            nc.sync.dma_start(g_out[:], g_in[:])
        else:
            nc.gpsimd.collective_compute(
                "AllReduce", mybir.AluOpType.add, replica_groups=groups,
                ins=[g_in[:]], outs=[g_out[:]])
        gf = spool.tile([1, F], FP32, tag="gf")
        nc.sync.dma_start(gf[:], g_out[:])
        gb_t = spool.tile([1, F], BF, tag="gb")
        nc.vector.tensor_copy(gb_t[:], gf[:])
        gT_ps = ptr.tile([P, P], BF, tag="tr")
        nc.tensor.transpose(out=gT_ps[:, 0:1], in_=gb_t[:],
                            identity=ident[0:1, 0:1])
        gT = spool.tile([P, 1], BF, tag="gTs")
        nc.any.tensor_copy(gT[:], gT_ps[:, 0:1])
        y_acc = spool.tile([1, 2], FP32, tag="yacc")
        for i in range(2):
            hid_ps = pnode.tile([P, NODE_COLS], FP32, tag="np")
            nc.tensor.matmul(hid_ps[0:1, 0:F], lhsT=gT[:],
                             rhs=W1[:, i * F:(i + 1) * F],
                             start=True, stop=True)
            hid = spool.tile([1, F], FP32, tag="hids")
            nc.vector.tensor_tensor(out=hid[:], in0=hid_ps[0:1, 0:F],
                                    in1=b1[:, i * F:(i + 1) * F],
                                    op=mybir.AluOpType.add)
            hidr = spool.tile([1, F], BF, tag="hidr")
            nc.vector.tensor_scalar(out=hidr[:], in0=hid[:], scalar1=0.0,
                                    scalar2=None, op0=mybir.AluOpType.max)
            hT_ps = ptr.tile([P, P], BF, tag="tr")
            nc.tensor.transpose(out=hT_ps[:, 0:1], in_=hidr[:],
                                identity=ident[0:1, 0:1])
            hT = spool.tile([P, 1], BF, tag="hTs")
            nc.any.tensor_copy(hT[:], hT_ps[:, 0:1])
            yp = pnode.tile([P, NODE_COLS], FP32, tag="np")
            nc.tensor.matmul(yp[0:1, 0:2], lhsT=hT[:],
                             rhs=W2ab[:, i * 2:i * 2 + 2],
                             start=True, stop=True)
            if i == 0:
                nc.vector.tensor_copy(y_acc[:], yp[0:1, 0:2])
            else:
                nc.vector.tensor_tensor(out=y_acc[:], in0=y_acc[:],
                                        in1=yp[0:1, 0:2],
                                        op=mybir.AluOpType.add)
        y_sb = spool.tile([1, 2], FP32, tag="ysb")
        nc.vector.tensor_tensor(out=y_sb[:], in0=y_acc[:], in1=b2[:],
                                op=mybir.AluOpType.add)
        nc.sync.dma_start(y[:], y_sb[:])
        if rep_cm is not None:
            rep_cm.__exit__(None, None, None)

    nc.finalize()
    return nc


# ------------------------------------------------------------------- driver
_CACHE = {}


def kernel(**inputs):
    prep = host_prep(inputs)
    key = (prep["nb"], prep["ktiles"], prep["npad"], prep["n_nodes"])
    if key not in _CACHE:
        _CACHE[key] = build_program(*key)
    nc = _CACHE[key]
    res = run_bass_kernel_spmd(nc, make_in_maps(prep), list(range(NCORES)))
    return res.results[0]["y"].astype(np.float32)
